# revision 1
# baseline (speedup 1.0000x reference)
"""Trainium2 Bass kernel for nn_CrossAttention (B=2, N=2048, C=1024, H=16, D=64).

Strategy: sequence-parallel SPMD over 8 NeuronCores. Core i owns 512 rows of
the flattened [B*N, C] token axis (cores 0-3 = batch 0, cores 4-7 = batch 1).

Per core:
  - load x_t/x_s slices (f32->bf16 cast during DMA), transpose to [C, T] via PE
  - q^T = W_q^T x^T   (k^T likewise)  -> transposed activations [dims, tokens]
  - v   = x W_v       (natural layout [tokens, dims])
  - AllGather k^T and v across the 4 cores of the same batch
  - attention with keys on PSUM partitions: S^T = k^T-chunk^T-stationary @ q^T,
    exp on ScalarE in [128, 2048] instructions (4 heads batched), then
    O^T = [V|1]^T @ P^T accumulated over key chunks (the ones column produces
    the softmax row sums for free), normalize with reciprocal * broadcast
  - fuse: out = a^T-chunks^T @ W_fuse + b_fuse, write local 512 output rows
"""

import sys

if "/opt/trn_rl_repo" not in sys.path:
    sys.path.insert(0, "/opt/trn_rl_repo")

import numpy as np

B, N, C, H, D = 2, 2048, 1024, 16, 64
NCORES = 8
T = (B * N) // NCORES          # 512 tokens per core
NT = N                         # 2048 keys per batch
P = 128
SCALE = D ** -0.5              # 0.125
KV_K_ELEMS = C * T             # 524288  (k^T shard: [1024 dims, 512 tok])
KV_V_ELEMS = T * C             # 524288  (v shard:   [512 tok, 1024 dims])
KV_ELEMS = KV_K_ELEMS + KV_V_ELEMS
GROUPS = [[0, 1, 2, 3], [4, 5, 6, 7]]

_CACHE = {}


def _build():
    import concourse.bass as bass
    import concourse.mybir as mybir
    import concourse.tile as tile
    from concourse import bacc
    from concourse.masks import make_identity

    f32 = mybir.dt.float32
    bf16 = mybir.dt.bfloat16

    nc = bacc.Bacc("TRN2", num_devices=NCORES, debug=False, enable_asserts=False)

    x_t = nc.dram_tensor("x_t", [T, C], f32, kind="ExternalInput").ap()
    x_s = nc.dram_tensor("x_s", [T, C], f32, kind="ExternalInput").ap()
    w_q = nc.dram_tensor("W_q", [C, C], f32, kind="ExternalInput").ap()
    w_kv = nc.dram_tensor("W_kv", [C, 2 * C], f32, kind="ExternalInput").ap()
    w_f = nc.dram_tensor("W_fuse", [C, C], f32, kind="ExternalInput").ap()
    b_f = nc.dram_tensor("b_fuse", [1, C], f32, kind="ExternalInput").ap()
    out = nc.dram_tensor("out", [T, C], f32, kind="ExternalOutput").ap()

    with tile.TileContext(nc) as tc:
        import contextlib

        with contextlib.ExitStack() as stk:
            consts = stk.enter_context(tc.tile_pool(name="consts", bufs=1))
            dram = stk.enter_context(tc.tile_pool(name="dram", bufs=1, space="DRAM"))

            identity = consts.tile([P, P], bf16, name="identity")
            make_identity(nc, identity)

            bias_b = consts.tile([P, C], f32, name="bias_b")
            nc.gpsimd.dma_start(out=bias_b, in_=b_f.to_broadcast([P, C]))

            # persistent activations
            qT = [consts.tile([P, T], bf16, name=f"qT{m}") for m in range(8)]
            aT = [consts.tile([P, T], bf16, name=f"aT{c}") for c in range(8)]
            wf = [consts.tile([P, C], bf16, name=f"wf{c}") for c in range(8)]

            k_in = dram.tile([KV_K_ELEMS], bf16, name="k_in")
            v_in = dram.tile([KV_V_ELEMS], bf16, name="v_in")
            k_out = dram.tile([4 * KV_K_ELEMS], bf16, name="k_out")
            v_out = dram.tile([4 * KV_V_ELEMS], bf16, name="v_out")

            # ---- phase A1: x_s -> k^T/v projections -> allgather (critical) ----
            with tc.tile_pool(name="pa1", bufs=1) as pa, \
                 tc.tile_pool(name="pa1_ps", bufs=3, space="PSUM") as pa_ps, \
                 tc.tile_pool(name="tp1_ps", bufs=3, space="PSUM") as tp_ps:

                xs_nat = [pa.tile([P, C], bf16, name=f"xs_nat{i}") for i in range(4)]
                for i in range(4):
                    nc.gpsimd.dma_start(out=xs_nat[i], in_=x_s[i * P:(i + 1) * P, :])
                wkv = [pa.tile([P, 2 * C], bf16, name=f"wkv{c}") for c in range(8)]
                for c in range(8):
                    nc.gpsimd.dma_start(out=wkv[c], in_=w_kv[c * P:(c + 1) * P, :])

                xsT = [pa.tile([P, T], bf16, name=f"xsT{c}") for c in range(8)]
                for i in range(4):          # token tile
                    for c in range(8):      # C chunk
                        pst = tp_ps.tile([P, P], bf16, name="pst")
                        nc.tensor.transpose(
                            pst, xs_nat[i][:, c * P:(c + 1) * P], identity)
                        nc.vector.tensor_copy(
                            out=xsT[c][:, i * P:(i + 1) * P], in_=pst)

                # k^T projection -> DRAM bounce for allgather
                k_in_v = k_in.rearrange("(m p t) -> m p t", m=8, p=P, t=T)
                for m in range(8):
                    ps = pa_ps.tile([P, T], f32, name="proj_ps")
                    for c in range(8):
                        nc.tensor.matmul(ps, wkv[c][:, m * P:(m + 1) * P], xsT[c],
                                         start=(c == 0), stop=(c == 7))
                    kT_loc = pa.tile([P, T], bf16, name="kT_loc", bufs=3)
                    nc.vector.tensor_copy(out=kT_loc, in_=ps)
                    nc.sync.dma_start(out=k_in_v[m], in_=kT_loc)

                # fire the k allgather as soon as k lands; it overlaps the
                # v projection, q projection, and the attention pipeline fill
                nc.gpsimd.collective_compute(
                    "AllGather", mybir.AluOpType.bypass, replica_groups=GROUPS,
                    ins=[k_in[:].opt()], outs=[k_out[:].opt()])

                # v projection (natural layout) -> DRAM bounce
                v_in_v = v_in.rearrange("(q p c) -> q p c", q=4, p=P, c=C)
                for tt in range(4):
                    v_loc = pa.tile([P, C], bf16, name="v_loc", bufs=3)
                    for nh in range(2):
                        ps = pa_ps.tile([P, 512], f32, name="proj_ps")
                        for c in range(8):
                            nc.tensor.matmul(
                                ps,
                                xsT[c][:, tt * P:(tt + 1) * P],
                                wkv[c][:, C + nh * 512:C + (nh + 1) * 512],
                                start=(c == 0), stop=(c == 7))
                        nc.vector.tensor_copy(
                            out=v_loc[:, nh * 512:(nh + 1) * 512], in_=ps)
                    nc.sync.dma_start(out=v_in_v[tt], in_=v_loc)

            nc.gpsimd.collective_compute(
                "AllGather", mybir.AluOpType.bypass, replica_groups=GROUPS,
                ins=[v_in[:].opt()], outs=[v_out[:].opt()])

            # ---- phase A2: x_t -> q^T projection (overlaps the collectives) ----
            with tc.tile_pool(name="pa2", bufs=1) as pa, \
                 tc.tile_pool(name="pa2_ps", bufs=3, space="PSUM") as pa_ps, \
                 tc.tile_pool(name="tp2_ps", bufs=3, space="PSUM") as tp_ps:

                xt_nat = [pa.tile([P, C], bf16, name=f"xt_nat{i}") for i in range(4)]
                for i in range(4):
                    nc.gpsimd.dma_start(out=xt_nat[i], in_=x_t[i * P:(i + 1) * P, :])
                wq = [pa.tile([P, C], bf16, name=f"wq{c}") for c in range(8)]
                for c in range(8):
                    nc.gpsimd.dma_start(out=wq[c], in_=w_q[c * P:(c + 1) * P, :])

                xtT = [pa.tile([P, T], bf16, name=f"xtT{c}") for c in range(8)]
                for i in range(4):
                    for c in range(8):
                        pst = tp_ps.tile([P, P], bf16, name="pst")
                        nc.tensor.transpose(
                            pst, xt_nat[i][:, c * P:(c + 1) * P], identity)
                        nc.vector.tensor_copy(
                            out=xtT[c][:, i * P:(i + 1) * P], in_=pst)

                # q^T projection: [128 qdims, T] tiles, accumulate over C chunks
                for m in range(8):
                    ps = pa_ps.tile([P, T], f32, name="proj_ps")
                    for c in range(8):
                        nc.tensor.matmul(ps, wq[c][:, m * P:(m + 1) * P], xtT[c],
                                         start=(c == 0), stop=(c == 7))
                    nc.vector.tensor_copy(out=qT[m], in_=ps)

            # ---------------- phase B: attention ----------------
            with tc.tile_pool(name="attn", bufs=1) as attn, \
                 tc.tile_pool(name="st_ps", bufs=1, space="PSUM") as st_ps, \
                 tc.tile_pool(name="ot_ps", bufs=1, space="PSUM") as ot_ps, \
                 tc.tile_pool(name="ptp", bufs=10) as ptp, \
                 tc.tile_pool(name="sm", bufs=4) as sm:

                # gathered k^T: [128 kdims, 2048 batch tokens] x 8 tiles
                kTf = [attn.tile([P, 4 * T], bf16, name=f"kTf{m}") for m in range(8)]
                k_out_v = k_out.rearrange(
                    "(r m p t) -> m p r t", r=4, m=8, p=P, t=T)
                for m in range(8):
                    nc.sync.dma_start(
                        out=kTf[m].rearrange("p (r t) -> p r t", r=4),
                        in_=k_out_v[m])

                # gathered v staged as [V_h | 1] blocks: [128 keys, 16 heads, 65].
                # Contiguous DMA into v_full, then GpSimd restripes on-chip
                # (a strided DMA straight into vp costs ~32k tiny descriptors).
                vp = [attn.tile([P, H, D + 1], bf16, name=f"vp{kt}")
                      for kt in range(16)]
                v_out_v = v_out.rearrange(
                    "(r q p c) -> r q p c", r=4, q=4, p=P, c=C)
                with tc.tile_pool(name="vfp", bufs=4) as vfp:
                    for kt in range(16):
                        v_full = vfp.tile([P, C], bf16, name="v_full")
                        nc.sync.dma_start(out=v_full, in_=v_out_v[kt // 4, kt % 4])
                        nc.gpsimd.memset(vp[kt], 1.0)
                        nc.gpsimd.tensor_copy(
                            out=vp[kt][:, :, 0:D],
                            in_=v_full.rearrange("p (h d) -> p h d", h=H))

                # W_fuse load rides the idle DMA lanes during attention
                for c in range(8):
                    nc.gpsimd.dma_start(out=wf[c], in_=w_f[c * P:(c + 1) * P, :])

                rdram = dram.tile([H * T], mybir.dt.float32, name="rdram")
                rdram_v = rdram.rearrange("(h t) -> h t", h=H)

                def emit_st(hq, kt):
                    # scores^T for 4 heads, row-packed pairs run concurrently
                    st = st_ps.tile([P, 4, T], mybir.dt.float32, name="st")
                    for i in range(4):
                        h = 4 * hq + i
                        sub = h % 2
                        nc.tensor.matmul(
                            st[:, i, :],
                            kTf[h // 2][sub * D:(sub + 1) * D,
                                        kt * P:(kt + 1) * P],
                            qT[h // 2][sub * D:(sub + 1) * D, :],
                            start=True, stop=True,
                            tile_position=(sub * D, 0))
                    return st

                for hq in range(4):          # head quads
                    ot = [ot_ps.tile([D + 1, T], mybir.dt.float32, name="ot",
                                     tag=f"ot{i}") for i in range(4)]
                    st = emit_st(hq, 0)
                    for kt in range(16):     # key chunks of 128
                        pt = ptp.tile([P, 4, T], bf16, name="pt")
                        nc.scalar.activation(
                            pt[:], st[:],
                            mybir.ActivationFunctionType.Exp, scale=SCALE)
                        # PE program order: next chunk's scores BEFORE this
                        # chunk's P@V, so S^T(kt+1) issues as soon as the exp
                        # frees the PSUM slot and the next exp isn't stuck
                        # behind P@V(kt).
                        if kt < 15:
                            st = emit_st(hq, kt + 1)
                        elif hq < 3:
                            pass  # next quad's first S^T emitted at loop top
                        for i in range(4):
                            h = 4 * hq + i
                            nc.tensor.matmul(
                                ot[i], vp[kt][:, h, :], pt[:, i, :],
                                start=(kt == 0), stop=(kt == 15))
                    # drain PSUM: unnormalized O^T -> aT (bf16), 1/rowsum -> DRAM
                    for i in range(4):
                        h = 4 * hq + i
                        nc.vector.tensor_copy(
                            out=aT[h // 2][(h % 2) * D:(h % 2 + 1) * D, :],
                            in_=ot[i][0:D, :])
                        rc = sm.tile([1, T], mybir.dt.float32, name="rc")
                        nc.vector.reciprocal(rc, ot[i][D:D + 1, :])
                        nc.sync.dma_start(out=rdram_v[h], in_=rc)

                # normalize: aT[h] *= 1/rowsum (partition-broadcast via DRAM bounce;
                # one [64, 8, T] DMA per partition-half instead of 16 small ones)
                rb_big = attn.tile([P, 8, T], mybir.dt.float32, name="rb_big")
                for half in range(2):
                    bcast = bass.AP(
                        tensor=rdram.tensor, offset=rdram.offset + half * T,
                        ap=[[0, D], [2 * T, 8], [1, T]])
                    nc.gpsimd.dma_start(
                        out=rb_big[half * D:(half + 1) * D, :, :], in_=bcast)
                for m in range(8):
                    nc.vector.tensor_mul(out=aT[m], in0=aT[m], in1=rb_big[:, m, :])

            # ---------------- phase C: fuse projection ----------------
            with tc.tile_pool(name="fu", bufs=4) as fu, \
                 tc.tile_pool(name="fu_ps", bufs=4, space="PSUM") as fu_ps:
                for tt in range(4):
                    for nh in range(2):
                        ps = fu_ps.tile([P, 512], mybir.dt.float32, name="fps")
                        for c in range(8):
                            nc.tensor.matmul(
                                ps, aT[c][:, tt * P:(tt + 1) * P],
                                wf[c][:, nh * 512:(nh + 1) * 512],
                                start=(c == 0), stop=(c == 7))
                        ob = fu.tile([P, 512], mybir.dt.float32, name="ob")
                        nc.vector.tensor_add(
                            out=ob, in0=ps, in1=bias_b[:, nh * 512:(nh + 1) * 512])
                        nc.sync.dma_start(
                            out=out[tt * P:(tt + 1) * P, nh * 512:(nh + 1) * 512],
                            in_=ob)

    nc.compile()
    return nc


def _get_nc():
    if "nc" not in _CACHE:
        _CACHE["nc"] = _build()
    return _CACHE["nc"]


def kernel(**inputs):
    nc = _get_nc()
    from concourse import bass_utils

    x_t = np.asarray(inputs["x_t"], dtype=np.float32).reshape(B * N, C)
    x_s = np.asarray(inputs["x_s"], dtype=np.float32).reshape(B * N, C)
    w_q = np.asarray(inputs["W_q"], dtype=np.float32)
    w_kv = np.asarray(inputs["W_kv"], dtype=np.float32)
    w_f = np.asarray(inputs["W_fuse"], dtype=np.float32)
    b_f = np.asarray(inputs["b_fuse"], dtype=np.float32).reshape(1, C)

    in_maps = []
    for i in range(NCORES):
        in_maps.append({
            "x_t": x_t[i * T:(i + 1) * T],
            "x_s": x_s[i * T:(i + 1) * T],
            "W_q": w_q,
            "W_kv": w_kv,
            "W_fuse": w_f,
            "b_fuse": b_f,
        })

    res = bass_utils.run_bass_kernel_spmd(nc, in_maps, core_ids=list(range(NCORES)))
    out = np.concatenate([res.results[i]["out"] for i in range(NCORES)], axis=0)
    return out.reshape(B, N, C).astype(np.float32)


if __name__ == "__main__":
    _build()
    print("build+compile OK")



# revision 4
# speedup vs baseline: 1.1103x; 1.1103x over previous
"""Trainium2 Bass kernel for nn_CrossAttention (B=2, N=2048, C=1024, H=16, D=64).

Strategy: sequence-parallel SPMD over 8 NeuronCores. Core i owns 512 rows of
the flattened [B*N, C] token axis (cores 0-3 = batch 0, cores 4-7 = batch 1).

Per core:
  - priority DMA: x_s + K-half of W_kv land first so the k^T projection and
    the k AllGather trigger as early as possible (collectives serialize on
    one CC stream, so the k AG must fire first and fast)
  - k^T = W_k^T x_s^T -> AllGather;  v = x_s W_v -> AllGather (right after)
  - q^T projection + W_fuse loads overlap the collectives
  - attention in 2-head groups: S^T = k^T-chunk-stationary @ q^T into
    triple-buffered PSUM so the ScalarE exp chain runs back-to-back;
    P@V packs the two heads onto disjoint PE column groups (concurrent),
    and softmax row sums come from M=1 ones-matmuls on col groups 0/1
    (also concurrent) -- no [V|1] restriping on GpSimd at all
  - per-pair normalization (reciprocal roundtrip through DRAM for the
    partition-broadcast) pipelined under later pairs' attention
  - fuse: out = a^T-chunks^T @ W_fuse + b_fuse
"""

import sys

if "/opt/trn_rl_repo" not in sys.path:
    sys.path.insert(0, "/opt/trn_rl_repo")

import numpy as np

B, N, C, H, D = 2, 2048, 1024, 16, 64
NCORES = 8
T = (B * N) // NCORES          # 512 tokens per core
P = 128
SCALE = D ** -0.5              # 0.125
KV_K_ELEMS = C * T             # 524288  (k^T shard: [1024 dims, 512 tok])
KV_V_ELEMS = T * C             # 524288  (v shard:   [512 tok, 1024 dims])
GROUPS = [[0, 1, 2, 3], [4, 5, 6, 7]]

# fp8 wire options (halve AllGather bytes; e4m3 quantization noise only)
F8_K = False   # k AG + kTf + qT in fp8e4, S^T matmul in fp8
F8_V = False   # v AG in fp8e4, upcast to bf16 on DVE after gather

_CACHE = {}


def _build():
    import concourse.bass as bass
    import concourse.mybir as mybir
    import concourse.tile as tile
    from concourse import bacc
    from concourse.masks import make_identity

    f32 = mybir.dt.float32
    bf16 = mybir.dt.bfloat16
    fp8 = mybir.dt.float8e4
    k_dt = fp8 if F8_K else bf16
    v_dt = fp8 if F8_V else bf16

    nc = bacc.Bacc("TRN2", num_devices=NCORES, debug=False, enable_asserts=False)

    x_t = nc.dram_tensor("x_t", [T, C], f32, kind="ExternalInput").ap()
    x_s = nc.dram_tensor("x_s", [T, C], f32, kind="ExternalInput").ap()
    w_q = nc.dram_tensor("W_q", [C, C], f32, kind="ExternalInput").ap()
    w_kv = nc.dram_tensor("W_kv", [C, 2 * C], f32, kind="ExternalInput").ap()
    w_f = nc.dram_tensor("W_fuse", [C, C], f32, kind="ExternalInput").ap()
    b_f = nc.dram_tensor("b_fuse", [1, C], f32, kind="ExternalInput").ap()
    out = nc.dram_tensor("out", [T, C], f32, kind="ExternalOutput").ap()

    with tile.TileContext(nc) as tc:
        import contextlib

        with contextlib.ExitStack() as stk:
            consts = stk.enter_context(tc.tile_pool(name="consts", bufs=1))
            dram = stk.enter_context(tc.tile_pool(name="dram", bufs=1, space="DRAM"))

            identity = consts.tile([P, P], bf16, name="identity")
            make_identity(nc, identity)

            bias_b = consts.tile([P, C], f32, name="bias_b")
            ones = consts.tile([P, 1], bf16, name="ones")
            nc.gpsimd.memset(ones, 1.0)

            # persistent activations
            qT = [consts.tile([P, T], k_dt, name=f"qT{m}") for m in range(8)]
            aT = [consts.tile([P, T], bf16, name=f"aT{g}") for g in range(8)]
            wf = [consts.tile([P, C], bf16, name=f"wf{c}") for c in range(8)]

            k_in = dram.tile([KV_K_ELEMS], k_dt, name="k_in")
            v_in = dram.tile([KV_V_ELEMS], v_dt, name="v_in")
            k_out = dram.tile([4 * KV_K_ELEMS], k_dt, name="k_out")
            v_out = dram.tile([4 * KV_V_ELEMS], v_dt, name="v_out")
            rdram = dram.tile([H * T], f32, name="rdram")
            rdram_v = rdram.rearrange("(h t) -> h t", h=H)

            # ---- phase A1: x_s -> k^T proj -> k AG (critical path) ----
            with tc.tile_pool(name="pa1", bufs=1) as pa, \
                 tc.tile_pool(name="pr_ps", bufs=3, space="PSUM") as pr_ps, \
                 tc.tile_pool(name="tp_ps", bufs=3, space="PSUM") as tp_ps:

                # priority order: x_s tiles, then W_kv K-half, then the rest
                xs_nat = [pa.tile([P, C], bf16, name=f"xs_nat{i}") for i in range(4)]
                for i in range(4):
                    nc.gpsimd.dma_start(out=xs_nat[i], in_=x_s[i * P:(i + 1) * P, :])
                wkvK = [pa.tile([P, C], bf16, name=f"wkvK{c}") for c in range(8)]
                for c in range(8):
                    nc.gpsimd.dma_start(out=wkvK[c], in_=w_kv[c * P:(c + 1) * P, 0:C])

                # x_s transposes, c-major so xsT[c] completes early
                xsT = [pa.tile([P, T], bf16, name=f"xsT{c}") for c in range(8)]
                for c in range(8):
                    for i in range(4):
                        pst = tp_ps.tile([P, P], bf16, name="pst")
                        nc.tensor.transpose(
                            pst, xs_nat[i][:, c * P:(c + 1) * P], identity)
                        eng = nc.vector if (c % 2 == 0) else nc.scalar
                        if eng is nc.vector:
                            eng.tensor_copy(
                                out=xsT[c][:, i * P:(i + 1) * P], in_=pst)
                        else:
                            eng.copy(out=xsT[c][:, i * P:(i + 1) * P], in_=pst)

                # k^T projection -> DRAM bounce for allgather
                k_in_v = k_in.rearrange("(m p t) -> m p t", m=8, p=P, t=T)
                for m in range(8):
                    ps = pr_ps.tile([P, T], f32, name="proj_ps")
                    for c in range(8):
                        nc.tensor.matmul(ps, wkvK[c][:, m * P:(m + 1) * P], xsT[c],
                                         start=(c == 0), stop=(c == 7))
                    kT_loc = pa.tile([P, T], k_dt, name="kT_loc", bufs=3)
                    nc.vector.tensor_copy(out=kT_loc, in_=ps)
                    nc.sync.dma_start(out=k_in_v[m], in_=kT_loc)

                nc.gpsimd.collective_compute(
                    "AllGather", mybir.AluOpType.bypass, replica_groups=GROUPS,
                    ins=[k_in[:].opt()], outs=[k_out[:].opt()])

                # ---- phase A2: v projection -> v AG ----
                wkvV = [pa.tile([P, C], bf16, name=f"wkvV{c}") for c in range(8)]
                for c in range(8):
                    nc.gpsimd.dma_start(out=wkvV[c], in_=w_kv[c * P:(c + 1) * P, C:2 * C])

                v_in_v = v_in.rearrange("(q p c) -> q p c", q=4, p=P, c=C)
                for tt in range(4):
                    v_loc = pa.tile([P, C], v_dt, name="v_loc", bufs=3)
                    for nh in range(2):
                        ps = pr_ps.tile([P, T], f32, name="proj_ps")
                        for c in range(8):
                            nc.tensor.matmul(
                                ps,
                                xsT[c][:, tt * P:(tt + 1) * P],
                                wkvV[c][:, nh * 512:(nh + 1) * 512],
                                start=(c == 0), stop=(c == 7))
                        nc.vector.tensor_copy(
                            out=v_loc[:, nh * 512:(nh + 1) * 512], in_=ps)
                    nc.sync.dma_start(out=v_in_v[tt], in_=v_loc)

                nc.gpsimd.collective_compute(
                    "AllGather", mybir.AluOpType.bypass, replica_groups=GROUPS,
                    ins=[v_in[:].opt()], outs=[v_out[:].opt()])

                # ---- phase A3: x_t -> q^T projection (overlaps the AGs) ----
                xt_nat = [pa.tile([P, C], bf16, name=f"xt_nat{i}") for i in range(4)]
                for i in range(4):
                    nc.gpsimd.dma_start(out=xt_nat[i], in_=x_t[i * P:(i + 1) * P, :])
                wq = [pa.tile([P, C], bf16, name=f"wq{c}") for c in range(8)]
                for c in range(8):
                    nc.gpsimd.dma_start(out=wq[c], in_=w_q[c * P:(c + 1) * P, :])

                xtT = [pa.tile([P, T], bf16, name=f"xtT{c}") for c in range(8)]
                for c in range(8):
                    for i in range(4):
                        pst = tp_ps.tile([P, P], bf16, name="pst")
                        nc.tensor.transpose(
                            pst, xt_nat[i][:, c * P:(c + 1) * P], identity)
                        if c % 2 == 0:
                            nc.vector.tensor_copy(
                                out=xtT[c][:, i * P:(i + 1) * P], in_=pst)
                        else:
                            nc.scalar.copy(
                                out=xtT[c][:, i * P:(i + 1) * P], in_=pst)

                for m in range(8):
                    ps = pr_ps.tile([P, T], f32, name="proj_ps")
                    for c in range(8):
                        nc.tensor.matmul(ps, wq[c][:, m * P:(m + 1) * P], xtT[c],
                                         start=(c == 0), stop=(c == 7))
                    nc.vector.tensor_copy(out=qT[m], in_=ps)

                # W_fuse + bias ride behind everything else on the DMA queues
                for c in range(8):
                    nc.gpsimd.dma_start(out=wf[c], in_=w_f[c * P:(c + 1) * P, :])
                nc.gpsimd.dma_start(out=bias_b, in_=b_f.to_broadcast([P, C]))

            # ---------------- phase B: attention ----------------
            with tc.tile_pool(name="attn", bufs=1) as attn, \
                 tc.tile_pool(name="st_ps", bufs=3, space="PSUM") as st_ps, \
                 tc.tile_pool(name="ot_ps", bufs=1, space="PSUM") as ot_ps, \
                 tc.tile_pool(name="rs_ps", bufs=1, space="PSUM") as rs_ps, \
                 tc.tile_pool(name="ptp", bufs=24) as ptp, \
                 tc.tile_pool(name="sm", bufs=4) as sm, \
                 tc.tile_pool(name="rbp", bufs=3) as rbp:

                # gathered k^T: [128 kdims, 2048 batch keys] x 8 tiles
                kTf = [attn.tile([P, 4 * T], k_dt, name=f"kTf{m}") for m in range(8)]
                k_out_v = k_out.rearrange(
                    "(r m p t) -> m p r t", r=4, m=8, p=P, t=T)
                for m in range(8):
                    nc.gpsimd.dma_start(
                        out=kTf[m].rearrange("p (r t) -> p r t", r=4),
                        in_=k_out_v[m])

                # gathered v: natural [128 keys, 1024 dims] per key tile
                v_out_v = v_out.rearrange(
                    "(r q p c) -> r q p c", r=4, q=4, p=P, c=C)
                vp = [attn.tile([P, C], bf16, name=f"vp{kt}") for kt in range(16)]
                if F8_V:
                    with tc.tile_pool(name="vp8p", bufs=4) as vp8p:
                        for kt in range(16):
                            vp8 = vp8p.tile([P, C], fp8, name="vp8")
                            nc.gpsimd.dma_start(
                                out=vp8, in_=v_out_v[kt // 4, kt % 4])
                            nc.vector.tensor_copy(out=vp[kt], in_=vp8)
                else:
                    for kt in range(16):
                        nc.gpsimd.dma_start(out=vp[kt], in_=v_out_v[kt // 4, kt % 4])

                def emit_st(g, kt):
                    # scores^T for heads 2g, 2g+1: row-packed, run concurrently
                    st = st_ps.tile([P, 2, T], f32, name="st")
                    for i in range(2):
                        h = 2 * g + i
                        nc.tensor.matmul(
                            st[:, i, :],
                            kTf[g][i * D:(i + 1) * D, kt * P:(kt + 1) * P],
                            qT[g][i * D:(i + 1) * D, :],
                            start=True, stop=True,
                            tile_position=(i * D, 0))
                    return st

                for g in range(8):           # head pairs
                    ot = ot_ps.tile([P, T], f32, name="ot")
                    rs = rs_ps.tile([33, T], f32, name="rs")
                    st = emit_st(g, 0)
                    for kt in range(16):     # key chunks of 128
                        pt = ptp.tile([P, 2, T], bf16, name="pt")
                        nc.scalar.activation(
                            pt[:], st[:],
                            mybir.ActivationFunctionType.Exp, scale=SCALE)
                        # next chunk's scores issue on PE before this chunk's
                        # P@V so the exp chain never waits on the PE
                        if kt < 15:
                            st = emit_st(g, kt + 1)
                        # P@V: the two heads on disjoint column groups
                        for i in range(2):
                            h = 2 * g + i
                            nc.tensor.matmul(
                                ot[i * D:(i + 1) * D, :],
                                vp[kt][:, h * D:(h + 1) * D], pt[:, i, :],
                                start=(kt == 0), stop=(kt == 15),
                                tile_position=(0, i * D))
                        # row sums via M=1 ones-matmuls on col groups 0/1
                        for i in range(2):
                            nc.tensor.matmul(
                                rs[i * 32:i * 32 + 1, :],
                                ones[:, 0:1], pt[:, i, :],
                                start=(kt == 0), stop=(kt == 15),
                                tile_position=(0, i * 32))
                    # drain: unnormalized O^T -> aT (bf16), rowsums -> recip -> DRAM
                    nc.vector.tensor_copy(out=aT[g], in_=ot)
                    for i in range(2):
                        rc = sm.tile([1, T], f32, name="rc")
                        nc.vector.reciprocal(rc, rs[i * 32:i * 32 + 1, :])
                        nc.sync.dma_start(out=rdram_v[2 * g + i], in_=rc)
                    # normalize via partition-broadcast DMA bounce
                    rb = rbp.tile([P, T], f32, name="rb")
                    for i in range(2):
                        bcast = bass.AP(
                            tensor=rdram.tensor,
                            offset=rdram.offset + (2 * g + i) * T,
                            ap=[[0, D], [1, T]])
                        nc.gpsimd.dma_start(
                            out=rb[i * D:(i + 1) * D, :], in_=bcast)
                    nc.vector.tensor_mul(out=aT[g], in0=aT[g], in1=rb)

            # ---------------- phase C: fuse projection ----------------
            with tc.tile_pool(name="fu", bufs=4) as fu, \
                 tc.tile_pool(name="fu_ps", bufs=4, space="PSUM") as fu_ps:
                for tt in range(4):
                    for nh in range(2):
                        ps = fu_ps.tile([P, 512], f32, name="fps")
                        for c in range(8):
                            nc.tensor.matmul(
                                ps, aT[c][:, tt * P:(tt + 1) * P],
                                wf[c][:, nh * 512:(nh + 1) * 512],
                                start=(c == 0), stop=(c == 7))
                        ob = fu.tile([P, 512], f32, name="ob")
                        nc.vector.tensor_add(
                            out=ob, in0=ps, in1=bias_b[:, nh * 512:(nh + 1) * 512])
                        nc.sync.dma_start(
                            out=out[tt * P:(tt + 1) * P, nh * 512:(nh + 1) * 512],
                            in_=ob)

    nc.compile()
    return nc


def _get_nc():
    if "nc" not in _CACHE:
        _CACHE["nc"] = _build()
    return _CACHE["nc"]


def kernel(**inputs):
    nc = _get_nc()
    from concourse import bass_utils

    x_t = np.asarray(inputs["x_t"], dtype=np.float32).reshape(B * N, C)
    x_s = np.asarray(inputs["x_s"], dtype=np.float32).reshape(B * N, C)
    w_q = np.asarray(inputs["W_q"], dtype=np.float32)
    w_kv = np.asarray(inputs["W_kv"], dtype=np.float32)
    w_f = np.asarray(inputs["W_fuse"], dtype=np.float32)
    b_f = np.asarray(inputs["b_fuse"], dtype=np.float32).reshape(1, C)

    in_maps = []
    for i in range(NCORES):
        in_maps.append({
            "x_t": x_t[i * T:(i + 1) * T],
            "x_s": x_s[i * T:(i + 1) * T],
            "W_q": w_q,
            "W_kv": w_kv,
            "W_fuse": w_f,
            "b_fuse": b_f,
        })

    res = bass_utils.run_bass_kernel_spmd(nc, in_maps, core_ids=list(range(NCORES)))
    out = np.concatenate([res.results[i]["out"] for i in range(NCORES)], axis=0)
    return out.reshape(B, N, C).astype(np.float32)


if __name__ == "__main__":
    _build()
    print("build+compile OK")


# revision 5
# speedup vs baseline: 1.2989x; 1.1699x over previous
"""Trainium2 Bass kernel for nn_CrossAttention (B=2, N=2048, C=1024, H=16, D=64).

Strategy: sequence-parallel SPMD over 8 NeuronCores. Core i owns 512 rows of
the flattened [B*N, C] token axis (cores 0-3 = batch 0, cores 4-7 = batch 1).

Per core:
  - DMA gating via tile_wait_until: only x_s + K-half of W_kv land first so
    the k^T projection and the first k AllGather fire at ~20us (collectives
    serialize on one CC stream, so their order/size sets the whole schedule)
  - collectives split and interleaved: kA (head dims 0-511) -> vA (v dims
    0-511) -> kB -> vB, so attention pairs 0-3 start while the rest gathers
  - attention in 2-head groups: S^T = k^T-chunk-stationary @ q^T into
    double-buffered PSUM so the ScalarE exp chain runs back-to-back;
    P@V packs the two heads onto disjoint PE column groups (concurrent),
    and softmax row sums come from M=1 ones-matmuls on col groups 0/1
    (also concurrent) -- no [V|1] restriping anywhere
  - per-pair normalization: one reciprocal over the whole rowsum bank,
    DRAM-bounce partition broadcast, pipelined under later pairs
  - fuse: out = a^T-chunks^T @ W_fuse + b_fuse
"""

import sys

if "/opt/trn_rl_repo" not in sys.path:
    sys.path.insert(0, "/opt/trn_rl_repo")

import numpy as np

B, N, C, H, D = 2, 2048, 1024, 16, 64
NCORES = 8
T = (B * N) // NCORES          # 512 tokens per core
P = 128
SCALE = D ** -0.5              # 0.125
HALF_K = C * T // 2            # 262144 (half of the k^T shard)
HALF_V = T * C // 2            # 262144 (half of the v shard)
GROUPS = [[0, 1, 2, 3], [4, 5, 6, 7]]

_CACHE = {}


def _build():
    import concourse.bass as bass
    import concourse.mybir as mybir
    import concourse.tile as tile
    from concourse import bacc
    from concourse.masks import make_identity

    f32 = mybir.dt.float32
    bf16 = mybir.dt.bfloat16

    nc = bacc.Bacc("TRN2", num_devices=NCORES, debug=False, enable_asserts=False)

    x_t = nc.dram_tensor("x_t", [T, C], f32, kind="ExternalInput").ap()
    x_s = nc.dram_tensor("x_s", [T, C], f32, kind="ExternalInput").ap()
    w_q = nc.dram_tensor("W_q", [C, C], f32, kind="ExternalInput").ap()
    w_kv = nc.dram_tensor("W_kv", [C, 2 * C], f32, kind="ExternalInput").ap()
    w_f = nc.dram_tensor("W_fuse", [C, C], f32, kind="ExternalInput").ap()
    b_f = nc.dram_tensor("b_fuse", [1, C], f32, kind="ExternalInput").ap()
    out = nc.dram_tensor("out", [T, C], f32, kind="ExternalOutput").ap()

    with tile.TileContext(nc) as tc:
        import contextlib

        with contextlib.ExitStack() as stk:
            consts = stk.enter_context(tc.tile_pool(name="consts", bufs=1))
            dram = stk.enter_context(tc.tile_pool(name="dram", bufs=1, space="DRAM"))

            identity = consts.tile([P, P], bf16, name="identity")
            make_identity(nc, identity)

            bias_b = consts.tile([P, C], f32, name="bias_b")
            ones = consts.tile([P, 1], bf16, name="ones")
            nc.gpsimd.memset(ones, 1.0)

            # persistent activations
            qT = [consts.tile([P, T], bf16, name=f"qT{m}") for m in range(8)]
            aT = [consts.tile([P, T], bf16, name=f"aT{g}") for g in range(8)]
            wf = [consts.tile([P, C], bf16, name=f"wf{c}") for c in range(8)]

            k_inA = dram.tile([HALF_K], bf16, name="k_inA")
            k_inB = dram.tile([HALF_K], bf16, name="k_inB")
            v_inA = dram.tile([HALF_V], bf16, name="v_inA")
            v_inB = dram.tile([HALF_V], bf16, name="v_inB")
            k_outA = dram.tile([4 * HALF_K], bf16, name="k_outA")
            k_outB = dram.tile([4 * HALF_K], bf16, name="k_outB")
            v_outA = dram.tile([4 * HALF_V], bf16, name="v_outA")
            v_outB = dram.tile([4 * HALF_V], bf16, name="v_outB")
            rdram = dram.tile([H * T], f32, name="rdram")
            rdram_v = rdram.rearrange("(h t) -> h t", h=H)

            # ---- phase A: projections + interleaved AllGathers ----
            with tc.tile_pool(name="pa1", bufs=1) as pa, \
                 tc.tile_pool(name="pr_ps", bufs=3, space="PSUM") as pr_ps, \
                 tc.tile_pool(name="tp_ps", bufs=3, space="PSUM") as tp_ps:

                # wave 0 (t=0): x_s + K-half of W_kv only
                xs_nat = [pa.tile([P, C], bf16, name=f"xs_nat{i}") for i in range(4)]
                for i in range(4):
                    nc.gpsimd.dma_start(out=xs_nat[i], in_=x_s[i * P:(i + 1) * P, :])
                wkvK = [pa.tile([P, C], bf16, name=f"wkvK{c}") for c in range(8)]
                for c in range(8):
                    nc.gpsimd.dma_start(out=wkvK[c], in_=w_kv[c * P:(c + 1) * P, 0:C])

                # wave 1 (~15us): x_t, W_q, V-half of W_kv
                xt_nat = [pa.tile([P, C], bf16, name=f"xt_nat{i}") for i in range(4)]
                wq = [pa.tile([P, C], bf16, name=f"wq{c}") for c in range(8)]
                wkvV = [pa.tile([P, C], bf16, name=f"wkvV{c}") for c in range(8)]
                with tc.tile_wait_until(0.015):
                    for i in range(4):
                        nc.gpsimd.dma_start(out=xt_nat[i], in_=x_t[i * P:(i + 1) * P, :])
                    for c in range(8):
                        nc.gpsimd.dma_start(out=wq[c], in_=w_q[c * P:(c + 1) * P, :])
                    for c in range(8):
                        nc.gpsimd.dma_start(
                            out=wkvV[c], in_=w_kv[c * P:(c + 1) * P, C:2 * C])

                # wave 2 (~45us): W_fuse + bias (needed only at the tail)
                with tc.tile_wait_until(0.045):
                    for c in range(8):
                        nc.gpsimd.dma_start(out=wf[c], in_=w_f[c * P:(c + 1) * P, :])
                    nc.gpsimd.dma_start(out=bias_b, in_=b_f.to_broadcast([P, C]))

                # x_s transposes, c-major so xsT[c] completes early
                xsT = [pa.tile([P, T], bf16, name=f"xsT{c}") for c in range(8)]
                for c in range(8):
                    for i in range(4):
                        pst = tp_ps.tile([P, P], bf16, name="pst")
                        nc.tensor.transpose(
                            pst, xs_nat[i][:, c * P:(c + 1) * P], identity)
                        if c % 2 == 0:
                            nc.vector.tensor_copy(
                                out=xsT[c][:, i * P:(i + 1) * P], in_=pst)
                        else:
                            nc.scalar.copy(
                                out=xsT[c][:, i * P:(i + 1) * P], in_=pst)

                # k^T projection -> DRAM bounce; kA fires after m=0..3
                k_inA_v = k_inA.rearrange("(m p t) -> m p t", m=4, p=P, t=T)
                k_inB_v = k_inB.rearrange("(m p t) -> m p t", m=4, p=P, t=T)
                for m in range(8):
                    ps = pr_ps.tile([P, T], f32, name="proj_ps")
                    for c in range(8):
                        nc.tensor.matmul(ps, wkvK[c][:, m * P:(m + 1) * P], xsT[c],
                                         start=(c == 0), stop=(c == 7))
                    kT_loc = pa.tile([P, T], bf16, name="kT_loc", bufs=3)
                    nc.vector.tensor_copy(out=kT_loc, in_=ps)
                    dst = k_inA_v[m] if m < 4 else k_inB_v[m - 4]
                    nc.sync.dma_start(out=dst, in_=kT_loc)
                    if m == 3:
                        nc.gpsimd.collective_compute(
                            "AllGather", mybir.AluOpType.bypass,
                            replica_groups=GROUPS,
                            ins=[k_inA[:].opt()], outs=[k_outA[:].opt()])

                # v projection (natural layout, split into dim halves)
                v_inA_v = v_inA.rearrange("(q p c) -> q p c", q=4, p=P, c=512)
                v_inB_v = v_inB.rearrange("(q p c) -> q p c", q=4, p=P, c=512)
                for tt in range(4):
                    for nh in range(2):
                        ps = pr_ps.tile([P, T], f32, name="proj_ps")
                        for c in range(8):
                            nc.tensor.matmul(
                                ps,
                                xsT[c][:, tt * P:(tt + 1) * P],
                                wkvV[c][:, nh * 512:(nh + 1) * 512],
                                start=(c == 0), stop=(c == 7))
                        v_loc = pa.tile([P, 512], bf16, name="v_loc", bufs=4)
                        nc.vector.tensor_copy(out=v_loc, in_=ps)
                        dstv = v_inA_v[tt] if nh == 0 else v_inB_v[tt]
                        nc.sync.dma_start(out=dstv, in_=v_loc)

                # CC stream order: kA, vA, kB, vB
                nc.gpsimd.collective_compute(
                    "AllGather", mybir.AluOpType.bypass, replica_groups=GROUPS,
                    ins=[v_inA[:].opt()], outs=[v_outA[:].opt()])
                nc.gpsimd.collective_compute(
                    "AllGather", mybir.AluOpType.bypass, replica_groups=GROUPS,
                    ins=[k_inB[:].opt()], outs=[k_outB[:].opt()])
                nc.gpsimd.collective_compute(
                    "AllGather", mybir.AluOpType.bypass, replica_groups=GROUPS,
                    ins=[v_inB[:].opt()], outs=[v_outB[:].opt()])

                # x_t transposes + q^T projection (overlaps the AGs)
                xtT = [pa.tile([P, T], bf16, name=f"xtT{c}") for c in range(8)]
                for c in range(8):
                    for i in range(4):
                        pst = tp_ps.tile([P, P], bf16, name="pst")
                        nc.tensor.transpose(
                            pst, xt_nat[i][:, c * P:(c + 1) * P], identity)
                        if c % 2 == 0:
                            nc.vector.tensor_copy(
                                out=xtT[c][:, i * P:(i + 1) * P], in_=pst)
                        else:
                            nc.scalar.copy(
                                out=xtT[c][:, i * P:(i + 1) * P], in_=pst)

                for m in range(8):
                    ps = pr_ps.tile([P, T], f32, name="proj_ps")
                    for c in range(8):
                        nc.tensor.matmul(ps, wq[c][:, m * P:(m + 1) * P], xtT[c],
                                         start=(c == 0), stop=(c == 7))
                    nc.vector.tensor_copy(out=qT[m], in_=ps)

            # ---------------- phase B: attention ----------------
            with tc.tile_pool(name="attn", bufs=1) as attn, \
                 tc.tile_pool(name="st_ps", bufs=2, space="PSUM") as st_ps, \
                 tc.tile_pool(name="ot_ps", bufs=1, space="PSUM") as ot_ps, \
                 tc.tile_pool(name="rs_ps", bufs=2, space="PSUM") as rs_ps, \
                 tc.tile_pool(name="ptp", bufs=26) as ptp, \
                 tc.tile_pool(name="sm", bufs=3) as sm, \
                 tc.tile_pool(name="rbp", bufs=3) as rbp:

                # gathered k^T: [128 kdims, 2048 batch keys] x 8 tiles
                kTf = [attn.tile([P, 4 * T], bf16, name=f"kTf{m}") for m in range(8)]
                k_outA_v = k_outA.rearrange(
                    "(r m p t) -> m p r t", r=4, m=4, p=P, t=T)
                k_outB_v = k_outB.rearrange(
                    "(r m p t) -> m p r t", r=4, m=4, p=P, t=T)
                for m in range(4):
                    nc.gpsimd.dma_start(
                        out=kTf[m].rearrange("p (r t) -> p r t", r=4),
                        in_=k_outA_v[m])
                # vpA between kTfA and kTfB on the queue (readiness order)
                v_outA_v = v_outA.rearrange(
                    "(r q p c) -> r q p c", r=4, q=4, p=P, c=512)
                v_outB_v = v_outB.rearrange(
                    "(r q p c) -> r q p c", r=4, q=4, p=P, c=512)
                vpA = [attn.tile([P, 512], bf16, name=f"vpA{kt}") for kt in range(16)]
                vpB = [attn.tile([P, 512], bf16, name=f"vpB{kt}") for kt in range(16)]
                for kt in range(16):
                    nc.gpsimd.dma_start(out=vpA[kt], in_=v_outA_v[kt // 4, kt % 4])
                for m in range(4, 8):
                    nc.gpsimd.dma_start(
                        out=kTf[m].rearrange("p (r t) -> p r t", r=4),
                        in_=k_outB_v[m - 4])
                for kt in range(16):
                    nc.gpsimd.dma_start(out=vpB[kt], in_=v_outB_v[kt // 4, kt % 4])

                def emit_st(g, kt):
                    # scores^T for heads 2g, 2g+1: row-packed, run concurrently
                    st = st_ps.tile([P, 2, T], f32, name="st")
                    for i in range(2):
                        nc.tensor.matmul(
                            st[:, i, :],
                            kTf[g][i * D:(i + 1) * D, kt * P:(kt + 1) * P],
                            qT[g][i * D:(i + 1) * D, :],
                            start=True, stop=True,
                            tile_position=(i * D, 0))
                    return st

                for g in range(8):           # head pairs
                    ot = ot_ps.tile([P, T], f32, name="ot")
                    rs = rs_ps.tile([33, T], f32, name="rs")
                    vp = vpA if g < 4 else vpB
                    hb = 0 if g < 4 else 8   # head base within the vp half
                    st = emit_st(g, 0)
                    for kt in range(16):     # key chunks of 128
                        pt = ptp.tile([P, 2, T], bf16, name="pt")
                        nc.scalar.activation(
                            pt[:], st[:],
                            mybir.ActivationFunctionType.Exp, scale=SCALE)
                        # next chunk's scores issue on PE before this chunk's
                        # P@V so the exp chain never waits on the PE
                        if kt < 15:
                            st = emit_st(g, kt + 1)
                        # P@V: the two heads on disjoint column groups
                        for i in range(2):
                            h = 2 * g + i - hb
                            nc.tensor.matmul(
                                ot[i * D:(i + 1) * D, :],
                                vp[kt][:, h * D:(h + 1) * D], pt[:, i, :],
                                start=(kt == 0), stop=(kt == 15),
                                tile_position=(0, i * D))
                        # row sums via M=1 ones-matmuls on col groups 0/1
                        for i in range(2):
                            nc.tensor.matmul(
                                rs[i * 32:i * 32 + 1, :],
                                ones[:, 0:1], pt[:, i, :],
                                start=(kt == 0), stop=(kt == 15),
                                tile_position=(0, i * 32))
                    # drain: unnormalized O^T -> aT (bf16); one reciprocal over
                    # the whole rowsum bank (rows 0 and 32 are the real data)
                    nc.vector.tensor_copy(out=aT[g], in_=ot)
                    rcb = sm.tile([33, T], f32, name="rcb")
                    nc.vector.reciprocal(rcb, rs)
                    nc.sync.dma_start(out=rdram_v[2 * g], in_=rcb[0:1, :])
                    nc.sync.dma_start(out=rdram_v[2 * g + 1], in_=rcb[32:33, :])
                    # normalize via partition-broadcast DMA bounce
                    rb = rbp.tile([P, T], f32, name="rb")
                    for i in range(2):
                        bcast = bass.AP(
                            tensor=rdram.tensor,
                            offset=rdram.offset + (2 * g + i) * T,
                            ap=[[0, D], [1, T]])
                        nc.gpsimd.dma_start(
                            out=rb[i * D:(i + 1) * D, :], in_=bcast)
                    nc.vector.tensor_mul(out=aT[g], in0=aT[g], in1=rb)

            # ---------------- phase C: fuse projection ----------------
            with tc.tile_pool(name="fu", bufs=4) as fu, \
                 tc.tile_pool(name="fu_ps", bufs=4, space="PSUM") as fu_ps:
                for tt in range(4):
                    for nh in range(2):
                        ps = fu_ps.tile([P, 512], f32, name="fps")
                        for c in range(8):
                            nc.tensor.matmul(
                                ps, aT[c][:, tt * P:(tt + 1) * P],
                                wf[c][:, nh * 512:(nh + 1) * 512],
                                start=(c == 0), stop=(c == 7))
                        ob = fu.tile([P, 512], f32, name="ob")
                        nc.vector.tensor_add(
                            out=ob, in0=ps, in1=bias_b[:, nh * 512:(nh + 1) * 512])
                        nc.sync.dma_start(
                            out=out[tt * P:(tt + 1) * P, nh * 512:(nh + 1) * 512],
                            in_=ob)

    nc.compile()
    return nc


def _get_nc():
    if "nc" not in _CACHE:
        _CACHE["nc"] = _build()
    return _CACHE["nc"]


def kernel(**inputs):
    nc = _get_nc()
    from concourse import bass_utils

    x_t = np.asarray(inputs["x_t"], dtype=np.float32).reshape(B * N, C)
    x_s = np.asarray(inputs["x_s"], dtype=np.float32).reshape(B * N, C)
    w_q = np.asarray(inputs["W_q"], dtype=np.float32)
    w_kv = np.asarray(inputs["W_kv"], dtype=np.float32)
    w_f = np.asarray(inputs["W_fuse"], dtype=np.float32)
    b_f = np.asarray(inputs["b_fuse"], dtype=np.float32).reshape(1, C)

    in_maps = []
    for i in range(NCORES):
        in_maps.append({
            "x_t": x_t[i * T:(i + 1) * T],
            "x_s": x_s[i * T:(i + 1) * T],
            "W_q": w_q,
            "W_kv": w_kv,
            "W_fuse": w_f,
            "b_fuse": b_f,
        })

    res = bass_utils.run_bass_kernel_spmd(nc, in_maps, core_ids=list(range(NCORES)))
    out = np.concatenate([res.results[i]["out"] for i in range(NCORES)], axis=0)
    return out.reshape(B, N, C).astype(np.float32)


if __name__ == "__main__":
    _build()
    print("build+compile OK")
